# revision 5
# baseline (speedup 1.0000x reference)
"""MoE top-2 routed FFN (B=4, S=2048, D=1024, H=2048, E=8) on 8 TRN2 NeuronCores.

Strategy (expert-parallel with two-slot load balancing):
  - Host computes the tiny gate (softmax top-2) and folds each token's combine
    coefficient into its activation as x~ = c^(1/3) * x  (the expert FFN
    relu(xW1)^2*(xW3) @ W2 is degree-3 positively homogeneous in x, so the
    scaled input yields exactly c * FFN(x); padded tokens use c = 0).
  - Each core runs TWO fixed-size expert slots (A: CA tokens, B: CB tokens),
    each with its own weight set.  The busiest expert splits across two cores'
    A slots, the lightest across their B slots, the rest take one core's A+B.
    This drops per-core work from ceil128(max_e L_e) to ~avg_e L_e tokens.
  - Per slot: phase 1 computes gT[h, tok] = relu(W1 xT)^2 * (W3 xT) in bf16;
    phase 2 is "flipped" (W2T tiles stationary, token columns moving) so token
    counts need no 128-padding: psO[d, tok] accumulates over 16 h-tiles.
    Output [d, tok] tiles store bf16; host transposes and scatter-adds.
  - Engines: PE matmuls; DVE phase-1 elementwise; Scalar drains psO->SBUF and
    issues output DMAs; Sync issues weight DMAs; GpSimd/Scalar issue x DMAs.
  - SBUF operand tiles are kept at <= ~4.6KB per-partition pitch: larger
    pitches (8KB+) were measured to slow PE/DVE instruction execution ~20%.
"""

import os
import sys

import numpy as np

if os.path.isdir("/opt/trn_rl_repo") and "/opt/trn_rl_repo" not in sys.path:
    sys.path.insert(0, "/opt/trn_rl_repo")

import ml_dtypes

import concourse.bacc as bacc
import concourse.mybir as mybir
from concourse.bass_utils import run_bass_kernel_spmd
from concourse.tile import TileContext

B, S, D, H, E = 4, 2048, 1024, 2048, 8
N = B * S
P = 128
KT = D // P   # 8 contraction tiles over D
MT = H // P   # 16 tiles over H
DT = D // P   # 8 output d-tiles (phase 2)
XC = 4        # k-tiles fused per x DMA chunk

F32 = mybir.dt.float32
BF16 = mybir.dt.bfloat16
BF16_NP = ml_dtypes.bfloat16

# Set by test harness to capture profiling info.
TRACE = False
LAST_RESULTS = None


def _token_groups(c):
    """Split [0, c) into moving-dim groups of at most 512, min size 32."""
    groups = []
    off, rem = 0, c
    while rem > 0:
        g = min(512, rem)
        if 0 < rem - g < 32:
            g = rem - 32
        groups.append((off, g))
        off += g
        rem -= g
    return groups


def build_kernel(CA, CB):
    nc = bacc.Bacc("TRN2", target_bir_lowering=False)

    HA = min(512, CA)
    TA = CA - HA
    HB = min(512, CB)
    TB = CB - HB
    NXC = KT // XC  # x chunks (2)

    slots_meta = []
    for nm, C, HD, TL in (("a", CA, HA, TA), ("b", CB, HB, TB)):
        d = {
            "C": C, "H": HD, "T": TL,
            "xh": nc.dram_tensor(f"xh{nm}", [NXC, P, XC * HD], BF16,
                                 kind="ExternalInput"),
            "w13": nc.dram_tensor(f"w13{nm}", [MT, P, 2 * KT * P], BF16,
                                  kind="ExternalInput"),
            "w2": nc.dram_tensor(f"w2{nm}", [MT, P, D], BF16,
                                 kind="ExternalInput"),
            "out": nc.dram_tensor(f"out{nm}", [DT, P, C], BF16,
                                  kind="ExternalOutput"),
        }
        if TL > 0:
            d["xt"] = nc.dram_tensor(f"xt{nm}", [NXC, P, XC * TL], BF16,
                                     kind="ExternalInput")
        slots_meta.append(d)

    with TileContext(nc) as tc:
        with (
            tc.tile_pool(name="x_pool", bufs=2) as x_pool,
            tc.tile_pool(name="g_pool", bufs=1) as g_pool,
            tc.tile_pool(name="w13_pool", bufs=4) as w13_pool,
            tc.tile_pool(name="w2_pool", bufs=2) as w2_pool,
            tc.tile_pool(name="tmp_pool", bufs=2) as tmp_pool,
            tc.tile_pool(name="ob_pool", bufs=3) as ob_pool,
            tc.tile_pool(name="const_pool", bufs=1) as const_pool,
            tc.tile_pool(name="psAB", bufs=2, space="PSUM") as psAB_pool,
            tc.tile_pool(name="psO", bufs=4, space="PSUM") as psO_pool,
        ):
            # --- PE warmup: flip the HAM clock gate (1.2->2.4GHz) and keep
            # the PE busy until the first real operands land. ---------------
            warm = const_pool.tile([P, 512], BF16, tag="warm")
            nc.any.memset(warm[:], 0.0)
            pswarm = psO_pool.tile([P, 512], F32, tag="psO", name="pswarm")
            NWARM = 10
            for i in range(NWARM):
                nc.tensor.matmul(pswarm[:], warm[:, :P], warm[:],
                                 start=(i == 0), stop=(i == NWARM - 1))
            warmsink = const_pool.tile([P, 1], F32, tag="warmsink")
            nc.vector.tensor_scalar_mul(warmsink[:], pswarm[:, :1], 0.0)

            # per-slot runtime state
            st = [dict(), dict()]

            def emit_w13_dma(si, m):
                S = slots_meta[si]
                t = w13_pool.tile([P, 2 * KT * P], BF16, tag="w13",
                                  name=f"w13_{si}_{m}")
                nc.sync.dma_start(t[:], S["w13"][m])
                st[si].setdefault("w13", {})[m] = t

            def emit_w2_dma(si, q):
                # quarter q: 4 of the 16 per-hk w2 tiles
                S = slots_meta[si]
                w2ts = st[si].setdefault("w2", [])
                for hk in range(4 * q, 4 * q + 4):
                    t = w2_pool.tile([P, D], BF16, tag=f"w2_{hk}",
                                     name=f"w2_{si}_{hk}")
                    nc.sync.dma_start(t[:], S["w2"][hk])
                    w2ts.append(t)

            def emit_x_dmas(si, part):
                S = slots_meta[si]
                HD, TL = S["H"], S["T"]
                if part == "h":
                    xhs = []
                    for c2 in range(NXC):
                        xh = x_pool.tile([P, XC * HD], BF16, tag=f"xh{c2}",
                                         name=f"xh_{si}_{c2}")
                        nc.gpsimd.dma_start(xh[:], S["xh"][c2])
                        xhs.append(xh)
                    st[si]["xh"] = xhs
                elif TL > 0:
                    xts = []
                    for c2 in range(NXC):
                        xt = x_pool.tile([P, XC * TL], BF16, tag=f"xl{c2}",
                                         name=f"xl_{si}_{c2}")
                        nc.scalar.dma_start(xt[:], S["xt"][c2])
                        xts.append(xt)
                    st[si]["xl"] = xts

            def xt_slice(si, k, g0, gw):
                S = slots_meta[si]
                HD, TL = S["H"], S["T"]
                c2, j = k // XC, k % XC
                if g0 < HD:
                    assert g0 + gw <= HD
                    return st[si]["xh"][c2][:, j * HD + g0:j * HD + g0 + gw]
                o = j * TL + g0 - HD
                return st[si]["xl"][c2][:, o:o + gw]

            def phase1_m(si, m):
                S = slots_meta[si]
                w13t = st[si]["w13"].pop(m)
                if m == 0:
                    gts = []
                    for j in range(MT):
                        gt = g_pool.tile([P, CA], BF16, tag=f"g{j}",
                                         name=f"g_{si}_{j}")
                        gts.append(gt)
                    st[si]["g"] = gts
                gt = st[si]["g"][m]
                for g0, gw in _token_groups(S["C"]):
                    psA = psAB_pool.tile([P, 512], F32, tag="psA",
                                         name=f"psA_{si}_{m}_{g0}")
                    psB = psAB_pool.tile([P, 512], F32, tag="psB",
                                         name=f"psB_{si}_{m}_{g0}")
                    for k in range(KT):
                        nc.tensor.matmul(
                            psA[:, :gw],
                            w13t[:, k * P:(k + 1) * P],
                            xt_slice(si, k, g0, gw),
                            start=(k == 0), stop=(k == KT - 1),
                        )
                    for k in range(KT):
                        nc.tensor.matmul(
                            psB[:, :gw],
                            w13t[:, KT * P + k * P:KT * P + (k + 1) * P],
                            xt_slice(si, k, g0, gw),
                            start=(k == 0), stop=(k == KT - 1),
                        )
                    r = tmp_pool.tile([P, 512], F32, tag="r",
                                      name=f"r_{si}_{m}_{g0}")
                    nc.vector.tensor_relu(r[:, :gw], psA[:, :gw])
                    t2 = tmp_pool.tile([P, 512], F32, tag="t2",
                                       name=f"t2_{si}_{m}_{g0}")
                    nc.vector.tensor_mul(t2[:, :gw], r[:, :gw], r[:, :gw])
                    nc.vector.tensor_mul(gt[:, g0:g0 + gw], t2[:, :gw],
                                         psB[:, :gw])

            def phase2_dt(si, dt):
                S = slots_meta[si]
                gts = st[si]["g"]
                w2ts = st[si]["w2"]
                C = S["C"]
                groups = _token_groups(C)
                psOs = []
                for g0, gw in groups:
                    psOs.append(psO_pool.tile([P, 512], F32, tag="psO",
                                              name=f"psO_{si}_{dt}_{g0}"))
                for hk in range(MT):
                    wsl = w2ts[hk][:, dt * P:(dt + 1) * P]
                    for (g0, gw), ps in zip(groups, psOs):
                        nc.tensor.matmul(ps[:, :gw], wsl,
                                         gts[hk][:, g0:g0 + gw],
                                         start=(hk == 0), stop=(hk == MT - 1))
                ob = ob_pool.tile([P, CA], BF16, tag="ob",
                                  name=f"ob_{si}_{dt}")
                for (g0, gw), ps in zip(groups, psOs):
                    nc.scalar.copy(ob[:, g0:g0 + gw], ps[:, :gw])
                nc.scalar.dma_start(S["out"][dt], ob[:, :C])

            # ---- emission ------------------------------------------------
            emit_w13_dma(0, 0)
            emit_x_dmas(0, "h")
            emit_w13_dma(0, 1)
            emit_x_dmas(0, "t")
            W2Q_AT = (3, 6, 9, 12)
            for m in range(MT):
                if m + 2 < MT:
                    emit_w13_dma(0, m + 2)
                if m in W2Q_AT:
                    emit_w2_dma(0, W2Q_AT.index(m))
                phase1_m(0, m)

            emit_w13_dma(1, 0)
            emit_x_dmas(1, "h")
            emit_w13_dma(1, 1)
            emit_x_dmas(1, "t")
            for dt in range(DT):
                for j in (2 * dt + 2, 2 * dt + 3):
                    if j < MT:
                        emit_w13_dma(1, j)
                if dt in (1, 3, 5, 7):
                    emit_w2_dma(1, (1, 3, 5, 7).index(dt))
                phase2_dt(0, dt)

            for m in range(MT):
                phase1_m(1, m)
            for dt in range(DT):
                phase2_dt(1, dt)

    if not nc.is_finalized():
        nc.finalize()
    return nc


def _slot_plan(loads):
    """Two fixed slots (a >= b) per core; returns (a, b, assignment).

    assignment: list per core of dicts {"A": (expert, lo, hi), "B": ...}
    where [lo, hi) indexes into that expert's routed-token list.
    """
    L = np.asarray(loads)
    order = np.argsort(-L, kind="stable")
    hi, lo = order[0], order[-1]
    mids = order[1:-1]
    a = int(-(-L[hi] // 2))
    b = int(max(-(-L[lo] // 2), max(L[m] for m in mids) - a))
    asn = [None] * E
    asn[0] = {"A": (hi, 0, a), "B": (lo, 0, b)}
    asn[1] = {"A": (hi, a, int(L[hi])), "B": (lo, b, int(L[lo]))}
    for j, e in enumerate(mids):
        cut = min(a, int(L[e]))
        asn[2 + j] = {"A": (e, 0, cut), "B": (e, cut, int(L[e]))}
    # validate
    for c in asn:
        eA, l0, h0 = c["A"]
        eB, l1, h1 = c["B"]
        assert 0 <= h0 - l0 <= a and 0 <= h1 - l1 <= b
    return a, b, asn


def kernel(x, W1, W2, W3, gate_w, gate_b):
    global LAST_RESULTS

    xf = np.ascontiguousarray(x.reshape(N, D).astype(np.float32, copy=False))

    # ---- gate: softmax + top-2 (tiny, done on host) ------------------------
    logits = xf @ gate_w.T.astype(np.float32) + gate_b.astype(np.float32)
    logits -= logits.max(axis=-1, keepdims=True)
    probs = np.exp(logits)
    probs /= probs.sum(axis=-1, keepdims=True)
    order = np.argsort(-probs, axis=-1, kind="stable")
    i1, i2 = order[:, 0], order[:, 1]
    ar = np.arange(N)
    p1, p2 = probs[ar, i1], probs[ar, i2]
    ps = p1 + p2
    c1, c2 = p1 / ps, p2 / ps

    idx_list, coef_list = [], []
    for e in range(E):
        m1 = i1 == e
        m2 = i2 == e
        ide = np.nonzero(m1 | m2)[0]
        ce = np.where(m1[ide], c1[ide], c2[ide]).astype(np.float32)
        idx_list.append(ide)
        coef_list.append(ce)

    CA, CB, asn = _slot_plan([len(i) for i in idx_list])

    # ---- per-core input packing -------------------------------------------
    wpack_cache = {}

    def wpack(e):
        if e not in wpack_cache:
            w1e = np.asarray(W1[e], np.float32)
            w3e = np.asarray(W3[e], np.float32)
            w2e = np.asarray(W2[e], np.float32)
            w1p = w1e.reshape(MT, P, KT, P).transpose(0, 3, 2, 1)
            w3p = w3e.reshape(MT, P, KT, P).transpose(0, 3, 2, 1)
            w13 = np.ascontiguousarray(
                np.concatenate(
                    [w1p.reshape(MT, P, KT * P), w3p.reshape(MT, P, KT * P)],
                    axis=2)).astype(BF16_NP)
            w2p = np.ascontiguousarray(w2e.T).reshape(MT, P, D).astype(BF16_NP)
            wpack_cache[e] = (w13, w2p)
        return wpack_cache[e]

    NXC = KT // XC

    def xpack(e, l0, h0, C):
        ide = idx_list[e][l0:h0]
        c3 = np.cbrt(coef_list[e][l0:h0]).astype(np.float32)
        xg = np.zeros((C, D), np.float32)
        xg[:h0 - l0] = xf[ide] * c3[:, None]
        HD = min(512, C)
        T3 = np.ascontiguousarray(xg.T).reshape(KT, P, C)
        # fuse XC k-tiles per chunk: chunk[c2][p, j*W + c] = T3[c2*XC+j, p, c]
        xh = np.ascontiguousarray(
            T3[:, :, :HD].reshape(NXC, XC, P, HD).transpose(0, 2, 1, 3)
        ).reshape(NXC, P, XC * HD)
        out = {"xh": xh.astype(BF16_NP)}
        if C > HD:
            TL = C - HD
            xt = np.ascontiguousarray(
                T3[:, :, HD:].reshape(NXC, XC, P, TL).transpose(0, 2, 1, 3)
            ).reshape(NXC, P, XC * TL)
            out["xt"] = xt.astype(BF16_NP)
        return out

    in_maps = []
    for c in range(E):
        m = {}
        for nm, C in (("a", CA), ("b", CB)):
            e, l0, h0 = asn[c]["A" if nm == "a" else "B"]
            w13, w2p = wpack(e)
            m[f"w13{nm}"] = w13
            m[f"w2{nm}"] = w2p
            xp = xpack(e, l0, h0, C)
            m[f"xh{nm}"] = xp["xh"]
            if "xt" in xp:
                m[f"xt{nm}"] = xp["xt"]
        in_maps.append(m)

    # ---- build + run on 8 cores -------------------------------------------
    nc = build_kernel(CA, CB)
    res = None
    last_exc = None
    for attempt in range(3):
        try:
            res = run_bass_kernel_spmd(
                nc, in_maps, core_ids=list(range(E)),
                trace=TRACE and attempt == 0,
            )
            break
        except Exception as exc:  # transient device wedge / trace plumbing
            last_exc = exc
    if res is None:
        raise last_exc
    LAST_RESULTS = res

    # ---- combine ----------------------------------------------------------
    out = np.zeros((N, D), np.float32)
    for c in range(E):
        for nm, C in (("a", CA), ("b", CB)):
            e, l0, h0 = asn[c]["A" if nm == "a" else "B"]
            if h0 <= l0:
                continue
            oe = res.results[c][f"out{nm}"].astype(np.float32)  # [DT, P, C]
            oe = oe.transpose(2, 0, 1).reshape(C, D)
            out[idx_list[e][l0:h0]] += oe[:h0 - l0]

    return out.reshape(B, S, D)
